# revision 33
# baseline (speedup 1.0000x reference)
"""Trainium2 Bass kernel for a SiamRPN-style depthwise-xcorr head.

Computation (per batch sample):
  k = relu(bn(conv3x3(kernel, wk)))      # (256,7,7)  -> (256,5,5)
  s = relu(bn(conv3x3(search, ws)))      # (256,31,31)-> (256,29,29)
  f = depthwise_xcorr(s, k)              # (256,25,25)
  f = relu(bn(conv1x1(f, w1)))
  out = conv1x1(f, w2) + b2              # (256,25,25)

Sharding: data-parallel over batch, 8 samples per NeuronCore x 8 cores.
BN (eval mode) is folded into the conv weights on the host; the per-channel
shift is applied as the ScalarE activation bias during PSUM eviction.
The 3x3 convs are 18 accumulated matmuls (9 taps x 2 cin tiles) over
shifted-window access patterns.  The depthwise xcorr runs on the tensor
engine as 25 accumulated matmuls whose stationary operand is a diagonal
matrix diag(k[:, tap]); all 25 diagonals for a channel tile are built in
one VectorE tensor_tensor op (broadcast access patterns over an identity
tile).  All matmuls use float32r (full PE rate at moving-dim >= 256),
which requires even-sized innermost runs on both the moving operand and
the PSUM destination - hence every row-window is padded to an even width
(29->30, 25->26, 5->6) and the padded garbage column is never evicted
(matmul garbage stays confined to its output column).
"""

import sys

if "/opt/trn_rl_repo" not in sys.path:
    sys.path.insert(0, "/opt/trn_rl_repo")

import numpy as np

import concourse.bacc as bacc
import concourse.mybir as mybir
import concourse.tile as tile
from concourse.bass_utils import run_bass_kernel_spmd

EPS = 1e-5
B, CIN, H, SK, SS, COUT = 64, 256, 256, 7, 31, 256
NCORES = 8
NB = B // NCORES            # samples per core
OS = SS - 2                 # 29: search conv output
OK = SK - 2                 # 5: kernel conv output
OX = OS - OK + 1            # 25: xcorr output
NPIX = OX * OX              # 625
SSP = SS + 1                # 32: padded search row width
SKP = SK + 3                # 10: padded kernel row width
OSP = OS + 1                # 30: padded search-conv out row width
OXP = OX + 1                # 26: padded xcorr out row width
OKP = OK + 1                # 6:  padded kernel-conv out (kf) row width
OKW = 8                     # kernel-conv matmul window width (N=8*5*8=320)
NKF = NB * OK * OKP         # 240

F32 = mybir.dt.float32
F32R = mybir.dt.float32r

# output-row chunks: each accumulated matmul's dst must sit inside one
# 512-f32 PSUM bank
S_CHUNKS = [(0, 15), (15, 14)]   # 15*30=450, 14*30=420 (both LDW-bound)
X_CHUNKS = [(0, 13), (13, 12)]   # 13*26=338, 12*26=312

_CACHED = {}


def _build_nc():
    nc = bacc.Bacc("TRN2", target_bir_lowering=False, debug=False,
                   num_devices=NCORES)

    xs_d = nc.dram_tensor("xs", [NB, CIN, SS, SSP], F32R, kind="ExternalInput")
    xk_d = nc.dram_tensor("xk", [2, 128, NB * SK * SKP], F32R,
                          kind="ExternalInput")
    ws_d = nc.dram_tensor("ws", [128, 9, 2, 256], F32R, kind="ExternalInput")
    wk_d = nc.dram_tensor("wk", [128, 9, 2, 256], F32R, kind="ExternalInput")
    w1_d = nc.dram_tensor("w1", [128, 2, 256], F32R, kind="ExternalInput")
    w2_d = nc.dram_tensor("w2", [128, 2, 256], F32R, kind="ExternalInput")
    bb_d = nc.dram_tensor("bb", [128, 8], F32, kind="ExternalInput")
    id_d = nc.dram_tensor("id128", [128, 128], F32R, kind="ExternalInput")
    y_d = nc.dram_tensor("y", [NB, COUT, NPIX], F32, kind="ExternalOutput")

    RELU = mybir.ActivationFunctionType.Relu
    IDENT = mybir.ActivationFunctionType.Identity
    COPY = mybir.ActivationFunctionType.Copy

    with tile.TileContext(nc) as tc:
        with (
            tc.tile_pool(name="wpool", bufs=1) as wpool,
            tc.tile_pool(name="xin", bufs=3) as xin,
            tc.tile_pool(name="smid", bufs=3) as smid,
            tc.tile_pool(name="dpool", bufs=2) as dpool,
            tc.tile_pool(name="fpool", bufs=2) as fpool,
            tc.tile_pool(name="opool", bufs=2) as opool,
            tc.tile_pool(name="ps_s", bufs=3, space="PSUM") as ps_s,
            tc.tile_pool(name="ps_x", bufs=3, space="PSUM") as ps_x,
            tc.tile_pool(name="ps_h", bufs=2, space="PSUM") as ps_h,
        ):
            # startup priority: sample-0 search inputs and the first ws tap
            # chunk land first, each on its own queue, so the first search
            # matmuls start as early as possible
            ws_t = wpool.tile([128, 9, 2, 256], F32R, tag="ws")
            bb_t = wpool.tile([128, 8], F32, tag="bb")
            id_t = wpool.tile([128, 128], F32R, tag="id")
            nc.gpsimd.dma_start(bb_t[:], bb_d[:])
            for tc3 in range(3):
                nc.sync.dma_start(ws_t[:, 3 * tc3:3 * (tc3 + 1), :, :],
                                  ws_d[:, 3 * tc3:3 * (tc3 + 1), :, :])
            wk_t = wpool.tile([128, 9, 2, 256], F32R, tag="wk")
            xk_t = [wpool.tile([128, NB, SK * SKP], F32R, tag=f"xk{j}",
                               name=f"xk{j}") for j in range(2)]
            for tc3 in range(3):
                nc.gpsimd.dma_start(wk_t[:, 3 * tc3:3 * (tc3 + 1), :, :],
                                    wk_d[:, 3 * tc3:3 * (tc3 + 1), :, :])
            for j in range(2):
                nc.gpsimd.dma_start(xk_t[j][:], xk_d[j, :, :])
            nc.gpsimd.dma_start(id_t[:], id_d[:])
            w1_t = wpool.tile([128, 2, 256], F32R, tag="w1")
            w2_t = wpool.tile([128, 2, 256], F32R, tag="w2")
            nc.gpsimd.dma_start(w1_t[:], w1_d[:])
            nc.gpsimd.dma_start(w2_t[:], w2_d[:])

            def bias(col):
                return bb_t[:, col:col + 1]


            # ---- kernel branch, all samples at once (N = 8*5*8 = 320) ----
            kf_t = [wpool.tile([128, NKF], F32, tag=f"kf{m}", name=f"kf{m}")
                    for m in range(2)]

            def kernel_conv():
                for m in range(2):
                    pk = ps_h.tile([128, 512], F32, tag="ph",
                                   name=f"pk{m}")
                    first = True
                    for t in range(9):
                        ky, kx = divmod(t, 3)
                        for j in range(2):
                            rhs = xk_t[j][:].rearrange(
                                "p s (a b) -> p s a b", a=SK, b=SKP)[
                                :, :, ky:ky + OK, kx:kx + OKW]
                            nc.tensor.matmul(
                                pk[:, 0:NB * OK * OKW],
                                wk_t[:, t, j, m * 128:(m + 1) * 128],
                                rhs, start=first, stop=(t == 8 and j == 1))
                            first = False
                    # compact 8-wide psum rows into the 6-wide kf layout
                    nc.scalar.activation(
                        kf_t[m][:].rearrange("p (s a b) -> p s a b",
                                             s=NB, a=OK, b=OKP),
                        pk[:, 0:NB * OK * OKW].rearrange(
                            "p (s a b) -> p s a b",
                            s=NB, a=OK, b=OKW)[:, :, :, 0:OKP],
                        RELU, bias=bias(2 + m))

            # ---- per-sample pipeline, software-pipelined emission:
            # xcorr+heads of sample s are emitted after the search conv of
            # sample s+1, so the PE always has independent matmul work while
            # the vector engine builds diagonals / accumulates rows 3-4 ----
            ss_all = {}

            def search_conv(s):
                xs_t = [xin.tile([128, SS, SSP], F32R, tag=f"xs{j}",
                                 name=f"xs{j}_{s}") for j in range(2)]
                for j in range(2):
                    nc.scalar.dma_start(
                        xs_t[j][:, 0:19, :],
                        xs_d[s, j * 128:(j + 1) * 128, 0:19, :])
                for j in range(2):
                    nc.scalar.dma_start(
                        xs_t[j][:, 19:SS, :],
                        xs_d[s, j * 128:(j + 1) * 128, 19:SS, :])
                ss_t = [smid.tile([128, OS, OSP], F32R, tag=f"ss{m}",
                                  name=f"ss{m}_{s}") for m in range(2)]
                for m in range(2):
                    for ci, (r0, nr) in enumerate(S_CHUNKS):
                        psm = ps_s.tile([128, 512], F32, tag="ps",
                                        name=f"psm{m}_{ci}_{s}")
                        first = True
                        for t in range(9):
                            ky, kx = divmod(t, 3)
                            for j in range(2):
                                rhs = xs_t[j][:, r0 + ky:r0 + ky + nr,
                                              kx:kx + OSP]
                                nc.tensor.matmul(
                                    psm[:, 0:nr * OSP],
                                    ws_t[:, t, j, m * 128:(m + 1) * 128],
                                    rhs, start=first,
                                    stop=(t == 8 and j == 1))
                                first = False
                        # evict full padded rows: the pad column only ever
                        # feeds other pad (garbage) columns downstream
                        nc.scalar.activation(
                            ss_t[m][:, r0:r0 + nr, :],
                            psm[:, 0:nr * OSP],
                            RELU, bias=bias(m))
                ss_all[s] = ss_t

            def xcorr_heads(s):
                ss_t = ss_all.pop(s)
                # xcorr: kernel rows 0-2 (15 taps) as diagonal matmuls on
                # the tensor engine; rows 3-4 (10 taps) as fused
                # multiply-accumulate chains on the vector engine.  For the
                # first and last samples (no neighbor work to hide the DVE
                # chain) all 25 taps run on the tensor engine instead.
                # taps 0..NTAP_PE-1 on PE, rest on VectorE; edge samples
                # lean on the PE (no neighbor work hides the DVE chain)
                NTAP_PE = 15 if s in (0, NB - 1) else 12
                ft_t = [fpool.tile([128, OX, OXP], F32R, tag=f"ft{j}",
                                   name=f"ft{j}_{s}") for j in range(2)]
                for j in range(2):
                    NT = OK * OKP
                    NDG = (OK - 2) * OKP
                    dg = dpool.tile([128, NDG, 128], F32R, tag=f"dg{j}",
                                    name=f"dg{j}_{s}")
                    # diagonals built on ScalarE (Copy with per-partition
                    # scale), only for the taps the PE runs
                    for t in range(NTAP_PE):
                        ky, kx = divmod(t, OK)
                        col = ky * OKP + kx
                        nc.scalar.activation(
                            dg[:, col, :], id_t[:], COPY,
                            scale=kf_t[j][:, s * NT + col:s * NT + col + 1])

                    acc = fpool.tile([128, OX, OXP], F32, tag=f"ac{j}",
                                     name=f"ac{j}_{s}")
                    first_acc = True
                    for t in range(NTAP_PE, OK * OK):
                        ky, kx = divmod(t, OK)
                        win = ss_t[j][:, ky:ky + OX, kx:kx + OXP]
                        kcol = kf_t[j][:, s * NT + ky * OKP + kx:
                                       s * NT + ky * OKP + kx + 1]
                        if first_acc:
                            nc.vector.tensor_scalar_mul(acc[:], win, kcol)
                            first_acc = False
                        else:
                            nc.vector.scalar_tensor_tensor(
                                acc[:], win, kcol, acc[:],
                                mybir.AluOpType.mult, mybir.AluOpType.add)

                    for ci, (r0, nr) in enumerate(X_CHUNKS):
                        psx = ps_x.tile([128, 512], F32, tag="px",
                                        name=f"psx{j}_{ci}_{s}")
                        for t in range(NTAP_PE):
                            ky, kx = divmod(t, OK)
                            rhs = ss_t[j][:, r0 + ky:r0 + ky + nr,
                                          kx:kx + OXP]
                            nc.tensor.matmul(
                                psx[:, 0:nr * OXP], dg[:, ky * OKP + kx, :],
                                rhs, start=(t == 0),
                                stop=(t == NTAP_PE - 1))
                        nc.vector.tensor_tensor(
                            ft_t[j][:, r0:r0 + nr, :],
                            psx[:, 0:nr * OXP].rearrange(
                                "p (a b) -> p a b", a=nr, b=OXP),
                            acc[:, r0:r0 + nr, :],
                            mybir.AluOpType.add)

                # 1x1 heads
                f2_t = [fpool.tile([128, OX, OXP], F32R, tag=f"f2{m}",
                                   name=f"f2{m}_{s}") for m in range(2)]
                for m in range(2):
                    for ci, (r0, nr) in enumerate(X_CHUNKS):
                        ps1 = ps_h.tile([128, 512], F32, tag="ph",
                                        name=f"ps1{m}_{ci}_{s}")
                        for j in range(2):
                            nc.tensor.matmul(
                                ps1[:, 0:nr * OXP],
                                w1_t[:, j, m * 128:(m + 1) * 128],
                                ft_t[j][:, r0:r0 + nr, :],
                                start=(j == 0), stop=(j == 1))
                        nc.scalar.activation(
                            f2_t[m][:, r0:r0 + nr, :],
                            ps1[:, 0:nr * OXP],
                            RELU, bias=bias(4 + m))

                for m in range(2):
                    # flat output tile -> fully contiguous DMA to DRAM
                    out_t = opool.tile([128, NPIX], F32, tag=f"o{m}",
                                       name=f"o{m}_{s}")
                    for ci, (r0, nr) in enumerate(X_CHUNKS):
                        ps2 = ps_h.tile([128, 512], F32, tag="ph",
                                        name=f"ps2{m}_{ci}_{s}")
                        for j in range(2):
                            nc.tensor.matmul(
                                ps2[:, 0:nr * OXP],
                                w2_t[:, j, m * 128:(m + 1) * 128],
                                f2_t[j][:, r0:r0 + nr, :],
                                start=(j == 0), stop=(j == 1))
                        nc.scalar.activation(
                            out_t[:, r0 * OX:(r0 + nr) * OX],
                            ps2[:, 0:nr * OXP].rearrange(
                                "p (a b) -> p a b", a=nr, b=OXP)[:, :, 0:OX],
                            IDENT, bias=bias(6 + m))
                    nc.gpsimd.dma_start(
                        y_d[s, m * 128:(m + 1) * 128, :], out_t[:])

            for s in range(NB):
                search_conv(s)
                if s == 0:
                    kernel_conv()
                if s >= 1:
                    xcorr_heads(s - 1)
            xcorr_heads(NB - 1)

    nc.compile()
    return nc


def _get_nc():
    if "nc" not in _CACHED:
        _CACHED["nc"] = _build_nc()
    return _CACHED["nc"]


def _fold_bn(w, g, b, m, v):
    scale = g / np.sqrt(v + EPS)
    return w * scale[:, None, None, None], (b - m * scale)


def _pack3x3(w):
    t = w.transpose(2, 3, 1, 0).reshape(9, 2, 128, 256)  # t, j, p, c
    return np.ascontiguousarray(t.transpose(2, 0, 1, 3).astype(np.float32))


def _pack1x1(w):
    t = w[:, :, 0, 0].T.reshape(2, 128, 256)             # j, p, c
    return np.ascontiguousarray(t.transpose(1, 0, 2).astype(np.float32))


def _make_in_maps(kernel, search, wk, gk, bk, mk, vk, ws, gs, bs, ms, vs,
                  w1, g1, b1, m1, v1, w2, b2):
    wk_f, bk_f = _fold_bn(wk, gk, bk, mk, vk)
    ws_f, bs_f = _fold_bn(ws, gs, bs, ms, vs)
    w1_f, b1_f = _fold_bn(w1, g1, b1, m1, v1)

    xs = np.zeros((B, CIN, SS, SSP), np.float32)
    xs[:, :, :, :SS] = search
    xkp = np.zeros((B, CIN, SK, SKP), np.float32)
    xkp[:, :, :, :SK] = kernel
    # [2, 128, NB*70]: partition line holds all samples of one core
    xkp = xkp.reshape(B, CIN, SK * SKP)

    # bias columns: [bs0, bs1, bk0, bk1, b10, b11, b20, b21]
    bb = np.stack([bs_f[:128], bs_f[128:], bk_f[:128], bk_f[128:],
                   b1_f[:128], b1_f[128:],
                   np.asarray(b2)[:128], np.asarray(b2)[128:]],
                  axis=1).astype(np.float32)

    common = dict(
        ws=_pack3x3(ws_f), wk=_pack3x3(wk_f),
        w1=_pack1x1(w1_f), w2=_pack1x1(w2),
        bb=np.ascontiguousarray(bb),
        id128=np.eye(128, dtype=np.float32),
    )
    in_maps = []
    for c in range(NCORES):
        sl = slice(c * NB, (c + 1) * NB)
        xk_core = xkp[sl].reshape(NB, 2, 128, SK * SKP)
        xk_core = np.ascontiguousarray(
            xk_core.transpose(1, 2, 0, 3).reshape(2, 128, NB * SK * SKP))
        in_maps.append(dict(xs=np.ascontiguousarray(xs[sl]),
                            xk=xk_core, **common))
    return in_maps


def kernel(**inputs):
    in_maps = _make_in_maps(**inputs)
    nc = _get_nc()
    res = run_bass_kernel_spmd(nc, in_maps, core_ids=list(range(NCORES)))
    out = np.concatenate([r["y"] for r in res.results], axis=0)
    return out.reshape(B, COUT, OX, OX).astype(np.float32)


# revision 34
# speedup vs baseline: 1.0103x; 1.0103x over previous
"""Trainium2 Bass kernel for a SiamRPN-style depthwise-xcorr head.

Computation (per batch sample):
  k = relu(bn(conv3x3(kernel, wk)))      # (256,7,7)  -> (256,5,5)
  s = relu(bn(conv3x3(search, ws)))      # (256,31,31)-> (256,29,29)
  f = depthwise_xcorr(s, k)              # (256,25,25)
  f = relu(bn(conv1x1(f, w1)))
  out = conv1x1(f, w2) + b2              # (256,25,25)

Sharding: data-parallel over batch, 8 samples per NeuronCore x 8 cores.
BN (eval mode) is folded into the conv weights on the host; the per-channel
shift is applied as the ScalarE activation bias during PSUM eviction.
The 3x3 convs are 18 accumulated matmuls (9 taps x 2 cin tiles) over
shifted-window access patterns.  The depthwise xcorr runs on the tensor
engine as 25 accumulated matmuls whose stationary operand is a diagonal
matrix diag(k[:, tap]); all 25 diagonals for a channel tile are built in
one VectorE tensor_tensor op (broadcast access patterns over an identity
tile).  All matmuls use float32r (full PE rate at moving-dim >= 256),
which requires even-sized innermost runs on both the moving operand and
the PSUM destination - hence every row-window is padded to an even width
(29->30, 25->26, 5->6) and the padded garbage column is never evicted
(matmul garbage stays confined to its output column).
"""

import sys

if "/opt/trn_rl_repo" not in sys.path:
    sys.path.insert(0, "/opt/trn_rl_repo")

import numpy as np

import concourse.bacc as bacc
import concourse.mybir as mybir
import concourse.tile as tile
from concourse.bass_utils import run_bass_kernel_spmd

EPS = 1e-5
B, CIN, H, SK, SS, COUT = 64, 256, 256, 7, 31, 256
NCORES = 8
NB = B // NCORES            # samples per core
OS = SS - 2                 # 29: search conv output
OK = SK - 2                 # 5: kernel conv output
OX = OS - OK + 1            # 25: xcorr output
NPIX = OX * OX              # 625
SSP = SS + 1                # 32: padded search row width
SKP = SK + 3                # 10: padded kernel row width
OSP = OS + 1                # 30: padded search-conv out row width
OXP = OX + 1                # 26: padded xcorr out row width
OKP = OK + 1                # 6:  padded kernel-conv out (kf) row width
OKW = 8                     # kernel-conv matmul window width (N=8*5*8=320)
NKF = NB * OK * OKP         # 240

F32 = mybir.dt.float32
F32R = mybir.dt.float32r

# output-row chunks: each accumulated matmul's dst must sit inside one
# 512-f32 PSUM bank
S_CHUNKS = [(0, 15), (15, 14)]   # 15*30=450, 14*30=420 (both LDW-bound)
X_CHUNKS = [(0, 13), (13, 12)]   # 13*26=338, 12*26=312

_CACHED = {}


def _build_nc():
    nc = bacc.Bacc("TRN2", target_bir_lowering=False, debug=False,
                   num_devices=NCORES)

    xs_d = nc.dram_tensor("xs", [NB, CIN, SS, SSP], F32R, kind="ExternalInput")
    xk_d = nc.dram_tensor("xk", [2, 128, NB * SK * SKP], F32R,
                          kind="ExternalInput")
    ws_d = nc.dram_tensor("ws", [128, 9, 2, 256], F32R, kind="ExternalInput")
    wk_d = nc.dram_tensor("wk", [128, 9, 2, 256], F32R, kind="ExternalInput")
    w1_d = nc.dram_tensor("w1", [128, 2, 256], F32R, kind="ExternalInput")
    w2_d = nc.dram_tensor("w2", [128, 2, 256], F32R, kind="ExternalInput")
    bb_d = nc.dram_tensor("bb", [128, 8], F32, kind="ExternalInput")
    id_d = nc.dram_tensor("id128", [128, 128], F32R, kind="ExternalInput")
    y_d = nc.dram_tensor("y", [NB, COUT, NPIX], F32, kind="ExternalOutput")

    RELU = mybir.ActivationFunctionType.Relu
    IDENT = mybir.ActivationFunctionType.Identity
    COPY = mybir.ActivationFunctionType.Copy

    with tile.TileContext(nc) as tc:
        with (
            tc.tile_pool(name="wpool", bufs=1) as wpool,
            tc.tile_pool(name="xin", bufs=3) as xin,
            tc.tile_pool(name="smid", bufs=3) as smid,
            tc.tile_pool(name="dpool", bufs=2) as dpool,
            tc.tile_pool(name="fpool", bufs=2) as fpool,
            tc.tile_pool(name="opool", bufs=2) as opool,
            tc.tile_pool(name="ps_s", bufs=3, space="PSUM") as ps_s,
            tc.tile_pool(name="ps_x", bufs=3, space="PSUM") as ps_x,
            tc.tile_pool(name="ps_h", bufs=2, space="PSUM") as ps_h,
        ):
            # startup priority: sample-0 search inputs and the first ws tap
            # chunk land first, each on its own queue, so the first search
            # matmuls start as early as possible
            ws_t = wpool.tile([128, 9, 2, 256], F32R, tag="ws")
            bb_t = wpool.tile([128, 8], F32, tag="bb")
            id_t = wpool.tile([128, 128], F32R, tag="id")
            nc.gpsimd.dma_start(bb_t[:], bb_d[:])
            nc.sync.dma_start(ws_t[:, 0:3, :, :], ws_d[:, 0:3, :, :])
            nc.gpsimd.dma_start(ws_t[:, 3:6, :, :], ws_d[:, 3:6, :, :])
            nc.sync.dma_start(ws_t[:, 6:9, :, :], ws_d[:, 6:9, :, :])
            wk_t = wpool.tile([128, 9, 2, 256], F32R, tag="wk")
            xk_t = [wpool.tile([128, NB, SK * SKP], F32R, tag=f"xk{j}",
                               name=f"xk{j}") for j in range(2)]
            for tc3 in range(3):
                nc.gpsimd.dma_start(wk_t[:, 3 * tc3:3 * (tc3 + 1), :, :],
                                    wk_d[:, 3 * tc3:3 * (tc3 + 1), :, :])
            for j in range(2):
                nc.gpsimd.dma_start(xk_t[j][:], xk_d[j, :, :])
            nc.gpsimd.dma_start(id_t[:], id_d[:])
            w1_t = wpool.tile([128, 2, 256], F32R, tag="w1")
            w2_t = wpool.tile([128, 2, 256], F32R, tag="w2")
            nc.gpsimd.dma_start(w1_t[:], w1_d[:])
            nc.gpsimd.dma_start(w2_t[:], w2_d[:])

            def bias(col):
                return bb_t[:, col:col + 1]


            # ---- kernel branch, all samples at once (N = 8*5*8 = 320) ----
            kf_t = [wpool.tile([128, NKF], F32, tag=f"kf{m}", name=f"kf{m}")
                    for m in range(2)]

            def kernel_conv():
                for m in range(2):
                    pk = ps_h.tile([128, 512], F32, tag="ph",
                                   name=f"pk{m}")
                    first = True
                    for t in range(9):
                        ky, kx = divmod(t, 3)
                        for j in range(2):
                            rhs = xk_t[j][:].rearrange(
                                "p s (a b) -> p s a b", a=SK, b=SKP)[
                                :, :, ky:ky + OK, kx:kx + OKW]
                            nc.tensor.matmul(
                                pk[:, 0:NB * OK * OKW],
                                wk_t[:, t, j, m * 128:(m + 1) * 128],
                                rhs, start=first, stop=(t == 8 and j == 1))
                            first = False
                    # compact 8-wide psum rows into the 6-wide kf layout
                    nc.scalar.activation(
                        kf_t[m][:].rearrange("p (s a b) -> p s a b",
                                             s=NB, a=OK, b=OKP),
                        pk[:, 0:NB * OK * OKW].rearrange(
                            "p (s a b) -> p s a b",
                            s=NB, a=OK, b=OKW)[:, :, :, 0:OKP],
                        RELU, bias=bias(2 + m))

            # ---- per-sample pipeline, software-pipelined emission:
            # xcorr+heads of sample s are emitted after the search conv of
            # sample s+1, so the PE always has independent matmul work while
            # the vector engine builds diagonals / accumulates rows 3-4 ----
            ss_all = {}

            def search_conv(s):
                xs_t = [xin.tile([128, SS, SSP], F32R, tag=f"xs{j}",
                                 name=f"xs{j}_{s}") for j in range(2)]
                for j in range(2):
                    nc.scalar.dma_start(
                        xs_t[j][:, 0:19, :],
                        xs_d[s, j * 128:(j + 1) * 128, 0:19, :])
                for j in range(2):
                    nc.scalar.dma_start(
                        xs_t[j][:, 19:SS, :],
                        xs_d[s, j * 128:(j + 1) * 128, 19:SS, :])
                ss_t = [smid.tile([128, OS, OSP], F32R, tag=f"ss{m}",
                                  name=f"ss{m}_{s}") for m in range(2)]
                for m in range(2):
                    for ci, (r0, nr) in enumerate(S_CHUNKS):
                        psm = ps_s.tile([128, 512], F32, tag="ps",
                                        name=f"psm{m}_{ci}_{s}")
                        first = True
                        for t in range(9):
                            ky, kx = divmod(t, 3)
                            for j in range(2):
                                rhs = xs_t[j][:, r0 + ky:r0 + ky + nr,
                                              kx:kx + OSP]
                                nc.tensor.matmul(
                                    psm[:, 0:nr * OSP],
                                    ws_t[:, t, j, m * 128:(m + 1) * 128],
                                    rhs, start=first,
                                    stop=(t == 8 and j == 1))
                                first = False
                        # evict full padded rows: the pad column only ever
                        # feeds other pad (garbage) columns downstream
                        nc.scalar.activation(
                            ss_t[m][:, r0:r0 + nr, :],
                            psm[:, 0:nr * OSP],
                            RELU, bias=bias(m))
                ss_all[s] = ss_t

            def xcorr_heads(s):
                ss_t = ss_all.pop(s)
                # xcorr: kernel rows 0-2 (15 taps) as diagonal matmuls on
                # the tensor engine; rows 3-4 (10 taps) as fused
                # multiply-accumulate chains on the vector engine.  For the
                # first and last samples (no neighbor work to hide the DVE
                # chain) all 25 taps run on the tensor engine instead.
                # taps 0..NTAP_PE-1 on PE, rest on VectorE; edge samples
                # lean on the PE (no neighbor work hides the DVE chain)
                NTAP_PE = 15 if s in (0, NB - 1) else 12
                ft_t = [fpool.tile([128, OX, OXP], F32R, tag=f"ft{j}",
                                   name=f"ft{j}_{s}") for j in range(2)]
                for j in range(2):
                    NT = OK * OKP
                    NDG = (OK - 2) * OKP
                    dg = dpool.tile([128, NDG, 128], F32R, tag=f"dg{j}",
                                    name=f"dg{j}_{s}")
                    # diagonals built on ScalarE (Copy with per-partition
                    # scale), only for the taps the PE runs
                    for t in range(NTAP_PE):
                        ky, kx = divmod(t, OK)
                        col = ky * OKP + kx
                        nc.scalar.activation(
                            dg[:, col, :], id_t[:], COPY,
                            scale=kf_t[j][:, s * NT + col:s * NT + col + 1])

                    acc = fpool.tile([128, OX, OXP], F32, tag=f"ac{j}",
                                     name=f"ac{j}_{s}")
                    first_acc = True
                    for t in range(NTAP_PE, OK * OK):
                        ky, kx = divmod(t, OK)
                        win = ss_t[j][:, ky:ky + OX, kx:kx + OXP]
                        kcol = kf_t[j][:, s * NT + ky * OKP + kx:
                                       s * NT + ky * OKP + kx + 1]
                        if first_acc:
                            nc.vector.tensor_scalar_mul(acc[:], win, kcol)
                            first_acc = False
                        else:
                            nc.vector.scalar_tensor_tensor(
                                acc[:], win, kcol, acc[:],
                                mybir.AluOpType.mult, mybir.AluOpType.add)

                    for ci, (r0, nr) in enumerate(X_CHUNKS):
                        psx = ps_x.tile([128, 512], F32, tag="px",
                                        name=f"psx{j}_{ci}_{s}")
                        for t in range(NTAP_PE):
                            ky, kx = divmod(t, OK)
                            rhs = ss_t[j][:, r0 + ky:r0 + ky + nr,
                                          kx:kx + OXP]
                            nc.tensor.matmul(
                                psx[:, 0:nr * OXP], dg[:, ky * OKP + kx, :],
                                rhs, start=(t == 0),
                                stop=(t == NTAP_PE - 1))
                        nc.vector.tensor_tensor(
                            ft_t[j][:, r0:r0 + nr, :],
                            psx[:, 0:nr * OXP].rearrange(
                                "p (a b) -> p a b", a=nr, b=OXP),
                            acc[:, r0:r0 + nr, :],
                            mybir.AluOpType.add)

                # 1x1 heads
                f2_t = [fpool.tile([128, OX, OXP], F32R, tag=f"f2{m}",
                                   name=f"f2{m}_{s}") for m in range(2)]
                for m in range(2):
                    for ci, (r0, nr) in enumerate(X_CHUNKS):
                        ps1 = ps_h.tile([128, 512], F32, tag="ph",
                                        name=f"ps1{m}_{ci}_{s}")
                        for j in range(2):
                            nc.tensor.matmul(
                                ps1[:, 0:nr * OXP],
                                w1_t[:, j, m * 128:(m + 1) * 128],
                                ft_t[j][:, r0:r0 + nr, :],
                                start=(j == 0), stop=(j == 1))
                        nc.scalar.activation(
                            f2_t[m][:, r0:r0 + nr, :],
                            ps1[:, 0:nr * OXP],
                            RELU, bias=bias(4 + m))

                for m in range(2):
                    # flat output tile -> fully contiguous DMA to DRAM
                    out_t = opool.tile([128, NPIX], F32, tag=f"o{m}",
                                       name=f"o{m}_{s}")
                    for ci, (r0, nr) in enumerate(X_CHUNKS):
                        ps2 = ps_h.tile([128, 512], F32, tag="ph",
                                        name=f"ps2{m}_{ci}_{s}")
                        for j in range(2):
                            nc.tensor.matmul(
                                ps2[:, 0:nr * OXP],
                                w2_t[:, j, m * 128:(m + 1) * 128],
                                f2_t[j][:, r0:r0 + nr, :],
                                start=(j == 0), stop=(j == 1))
                        nc.scalar.activation(
                            out_t[:, r0 * OX:(r0 + nr) * OX],
                            ps2[:, 0:nr * OXP].rearrange(
                                "p (a b) -> p a b", a=nr, b=OXP)[:, :, 0:OX],
                            IDENT, bias=bias(6 + m))
                    nc.gpsimd.dma_start(
                        y_d[s, m * 128:(m + 1) * 128, :], out_t[:])

            for s in range(NB):
                search_conv(s)
                if s == 0:
                    kernel_conv()
                if s >= 1:
                    xcorr_heads(s - 1)
            xcorr_heads(NB - 1)

    nc.compile()
    return nc


def _get_nc():
    if "nc" not in _CACHED:
        _CACHED["nc"] = _build_nc()
    return _CACHED["nc"]


def _fold_bn(w, g, b, m, v):
    scale = g / np.sqrt(v + EPS)
    return w * scale[:, None, None, None], (b - m * scale)


def _pack3x3(w):
    t = w.transpose(2, 3, 1, 0).reshape(9, 2, 128, 256)  # t, j, p, c
    return np.ascontiguousarray(t.transpose(2, 0, 1, 3).astype(np.float32))


def _pack1x1(w):
    t = w[:, :, 0, 0].T.reshape(2, 128, 256)             # j, p, c
    return np.ascontiguousarray(t.transpose(1, 0, 2).astype(np.float32))


def _make_in_maps(kernel, search, wk, gk, bk, mk, vk, ws, gs, bs, ms, vs,
                  w1, g1, b1, m1, v1, w2, b2):
    wk_f, bk_f = _fold_bn(wk, gk, bk, mk, vk)
    ws_f, bs_f = _fold_bn(ws, gs, bs, ms, vs)
    w1_f, b1_f = _fold_bn(w1, g1, b1, m1, v1)

    xs = np.zeros((B, CIN, SS, SSP), np.float32)
    xs[:, :, :, :SS] = search
    xkp = np.zeros((B, CIN, SK, SKP), np.float32)
    xkp[:, :, :, :SK] = kernel
    # [2, 128, NB*70]: partition line holds all samples of one core
    xkp = xkp.reshape(B, CIN, SK * SKP)

    # bias columns: [bs0, bs1, bk0, bk1, b10, b11, b20, b21]
    bb = np.stack([bs_f[:128], bs_f[128:], bk_f[:128], bk_f[128:],
                   b1_f[:128], b1_f[128:],
                   np.asarray(b2)[:128], np.asarray(b2)[128:]],
                  axis=1).astype(np.float32)

    common = dict(
        ws=_pack3x3(ws_f), wk=_pack3x3(wk_f),
        w1=_pack1x1(w1_f), w2=_pack1x1(w2),
        bb=np.ascontiguousarray(bb),
        id128=np.eye(128, dtype=np.float32),
    )
    in_maps = []
    for c in range(NCORES):
        sl = slice(c * NB, (c + 1) * NB)
        xk_core = xkp[sl].reshape(NB, 2, 128, SK * SKP)
        xk_core = np.ascontiguousarray(
            xk_core.transpose(1, 2, 0, 3).reshape(2, 128, NB * SK * SKP))
        in_maps.append(dict(xs=np.ascontiguousarray(xs[sl]),
                            xk=xk_core, **common))
    return in_maps


def kernel(**inputs):
    in_maps = _make_in_maps(**inputs)
    nc = _get_nc()
    res = run_bass_kernel_spmd(nc, in_maps, core_ids=list(range(NCORES)))
    out = np.concatenate([r["y"] for r in res.results], axis=0)
    return out.reshape(B, COUT, OX, OX).astype(np.float32)


# revision 35
# speedup vs baseline: 1.0333x; 1.0228x over previous
"""Trainium2 Bass kernel for a SiamRPN-style depthwise-xcorr head.

Computation (per batch sample):
  k = relu(bn(conv3x3(kernel, wk)))      # (256,7,7)  -> (256,5,5)
  s = relu(bn(conv3x3(search, ws)))      # (256,31,31)-> (256,29,29)
  f = depthwise_xcorr(s, k)              # (256,25,25)
  f = relu(bn(conv1x1(f, w1)))
  out = conv1x1(f, w2) + b2              # (256,25,25)

Sharding: data-parallel over batch, 8 samples per NeuronCore x 8 cores.
BN (eval mode) is folded into the conv weights on the host; the per-channel
shift is applied as the ScalarE activation bias during PSUM eviction.
The 3x3 convs are 18 accumulated matmuls (9 taps x 2 cin tiles) over
shifted-window access patterns.  The depthwise xcorr runs on the tensor
engine as 25 accumulated matmuls whose stationary operand is a diagonal
matrix diag(k[:, tap]); all 25 diagonals for a channel tile are built in
one VectorE tensor_tensor op (broadcast access patterns over an identity
tile).  All matmuls use float32r (full PE rate at moving-dim >= 256),
which requires even-sized innermost runs on both the moving operand and
the PSUM destination - hence every row-window is padded to an even width
(29->30, 25->26, 5->6) and the padded garbage column is never evicted
(matmul garbage stays confined to its output column).
"""

import sys

if "/opt/trn_rl_repo" not in sys.path:
    sys.path.insert(0, "/opt/trn_rl_repo")

import numpy as np

import concourse.bacc as bacc
import concourse.mybir as mybir
import concourse.tile as tile
from concourse.bass_utils import run_bass_kernel_spmd

EPS = 1e-5
B, CIN, H, SK, SS, COUT = 64, 256, 256, 7, 31, 256
NCORES = 8
NB = B // NCORES            # samples per core
OS = SS - 2                 # 29: search conv output
OK = SK - 2                 # 5: kernel conv output
OX = OS - OK + 1            # 25: xcorr output
NPIX = OX * OX              # 625
SSP = SS + 1                # 32: padded search row width
SKP = SK + 3                # 10: padded kernel row width
OSP = OS + 1                # 30: padded search-conv out row width
OXP = OX + 1                # 26: padded xcorr out row width
OKP = OK + 1                # 6:  padded kernel-conv out (kf) row width
OKW = 8                     # kernel-conv matmul window width (N=8*5*8=320)
NKF = NB * OK * OKP         # 240

F32 = mybir.dt.float32
F32R = mybir.dt.float32r

# output-row chunks: each accumulated matmul's dst must sit inside one
# 512-f32 PSUM bank
S_CHUNKS = [(0, 15), (15, 14)]   # 15*30=450, 14*30=420 (both LDW-bound)
X_CHUNKS = [(0, 13), (13, 12)]   # 13*26=338, 12*26=312

_CACHED = {}


def _build_nc():
    nc = bacc.Bacc("TRN2", target_bir_lowering=False, debug=False,
                   num_devices=NCORES)

    xs_d = nc.dram_tensor("xs", [NB, CIN, SS, SSP], F32R, kind="ExternalInput")
    xk_d = nc.dram_tensor("xk", [2, 128, NB * SK * SKP], F32R,
                          kind="ExternalInput")
    ws_d = nc.dram_tensor("ws", [128, 9, 2, 256], F32R, kind="ExternalInput")
    wk_d = nc.dram_tensor("wk", [128, 9, 2, 256], F32R, kind="ExternalInput")
    w1_d = nc.dram_tensor("w1", [128, 2, 256], F32R, kind="ExternalInput")
    w2_d = nc.dram_tensor("w2", [128, 2, 256], F32R, kind="ExternalInput")
    bb_d = nc.dram_tensor("bb", [128, 8], F32, kind="ExternalInput")
    id_d = nc.dram_tensor("id128", [128, 128], F32R, kind="ExternalInput")
    y_d = nc.dram_tensor("y", [NB, COUT, NPIX], F32, kind="ExternalOutput")

    RELU = mybir.ActivationFunctionType.Relu
    IDENT = mybir.ActivationFunctionType.Identity
    COPY = mybir.ActivationFunctionType.Copy

    with tile.TileContext(nc) as tc:
        with (
            tc.tile_pool(name="wpool", bufs=1) as wpool,
            tc.tile_pool(name="xin", bufs=3) as xin,
            tc.tile_pool(name="smid", bufs=3) as smid,
            tc.tile_pool(name="dpool", bufs=2) as dpool,
            tc.tile_pool(name="fpool", bufs=2) as fpool,
            tc.tile_pool(name="opool", bufs=2) as opool,
            tc.tile_pool(name="ps_s", bufs=3, space="PSUM") as ps_s,
            tc.tile_pool(name="ps_x", bufs=3, space="PSUM") as ps_x,
            tc.tile_pool(name="ps_h", bufs=2, space="PSUM") as ps_h,
        ):
            # startup priority: sample-0 search inputs and the first ws tap
            # chunk land first, each on its own queue, so the first search
            # matmuls start as early as possible
            ws_t = wpool.tile([128, 9, 2, 256], F32R, tag="ws")
            bb_t = wpool.tile([128, 8], F32, tag="bb")
            id_t = wpool.tile([128, 128], F32R, tag="id")
            nc.gpsimd.dma_start(bb_t[:], bb_d[:])
            nc.sync.dma_start(ws_t[:, 0:3, :, :], ws_d[:, 0:3, :, :])
            nc.gpsimd.dma_start(ws_t[:, 3:6, :, :], ws_d[:, 3:6, :, :])
            nc.sync.dma_start(ws_t[:, 6:9, :, :], ws_d[:, 6:9, :, :])
            wk_t = wpool.tile([128, 9, 2, 256], F32R, tag="wk")
            xk_t = [wpool.tile([128, NB, SK * SKP], F32R, tag=f"xk{j}",
                               name=f"xk{j}") for j in range(2)]
            for tc3 in range(3):
                nc.gpsimd.dma_start(wk_t[:, 3 * tc3:3 * (tc3 + 1), :, :],
                                    wk_d[:, 3 * tc3:3 * (tc3 + 1), :, :])
            for j in range(2):
                nc.gpsimd.dma_start(xk_t[j][:], xk_d[j, :, :])
            nc.gpsimd.dma_start(id_t[:], id_d[:])
            w1_t = wpool.tile([128, 2, 256], F32R, tag="w1")
            w2_t = wpool.tile([128, 2, 256], F32R, tag="w2")
            nc.gpsimd.dma_start(w1_t[:], w1_d[:])
            nc.gpsimd.dma_start(w2_t[:], w2_d[:])

            def bias(col):
                return bb_t[:, col:col + 1]


            # ---- kernel branch, all samples at once (N = 8*5*8 = 320) ----
            kf_t = [wpool.tile([128, NKF], F32, tag=f"kf{m}", name=f"kf{m}")
                    for m in range(2)]

            def kernel_conv():
                for m in range(2):
                    pk = ps_h.tile([128, 512], F32, tag="ph",
                                   name=f"pk{m}")
                    first = True
                    for t in range(9):
                        ky, kx = divmod(t, 3)
                        for j in range(2):
                            rhs = xk_t[j][:].rearrange(
                                "p s (a b) -> p s a b", a=SK, b=SKP)[
                                :, :, ky:ky + OK, kx:kx + OKW]
                            nc.tensor.matmul(
                                pk[:, 0:NB * OK * OKW],
                                wk_t[:, t, j, m * 128:(m + 1) * 128],
                                rhs, start=first, stop=(t == 8 and j == 1))
                            first = False
                    # compact 8-wide psum rows into the 6-wide kf layout
                    nc.scalar.activation(
                        kf_t[m][:].rearrange("p (s a b) -> p s a b",
                                             s=NB, a=OK, b=OKP),
                        pk[:, 0:NB * OK * OKW].rearrange(
                            "p (s a b) -> p s a b",
                            s=NB, a=OK, b=OKW)[:, :, :, 0:OKP],
                        RELU, bias=bias(2 + m))

            # ---- per-sample pipeline, software-pipelined emission:
            # xcorr+heads of sample s are emitted after the search conv of
            # sample s+1, so the PE always has independent matmul work while
            # the vector engine builds diagonals / accumulates rows 3-4 ----
            ss_all = {}

            def search_conv(s):
                xs_t = [xin.tile([128, SS, SSP], F32R, tag=f"xs{j}",
                                 name=f"xs{j}_{s}") for j in range(2)]
                for j in range(2):
                    nc.scalar.dma_start(
                        xs_t[j][:, 0:19, :],
                        xs_d[s, j * 128:(j + 1) * 128, 0:19, :])
                for j in range(2):
                    nc.scalar.dma_start(
                        xs_t[j][:, 19:SS, :],
                        xs_d[s, j * 128:(j + 1) * 128, 19:SS, :])
                ss_t = [smid.tile([128, OS, OSP], F32R, tag=f"ss{m}",
                                  name=f"ss{m}_{s}") for m in range(2)]
                for m in range(2):
                    for ci, (r0, nr) in enumerate(S_CHUNKS):
                        psm = ps_s.tile([128, 512], F32, tag="ps",
                                        name=f"psm{m}_{ci}_{s}")
                        first = True
                        for t in range(9):
                            ky, kx = divmod(t, 3)
                            for j in range(2):
                                rhs = xs_t[j][:, r0 + ky:r0 + ky + nr,
                                              kx:kx + OSP]
                                nc.tensor.matmul(
                                    psm[:, 0:nr * OSP],
                                    ws_t[:, t, j, m * 128:(m + 1) * 128],
                                    rhs, start=first,
                                    stop=(t == 8 and j == 1))
                                first = False
                        # evict full padded rows: the pad column only ever
                        # feeds other pad (garbage) columns downstream
                        nc.scalar.activation(
                            ss_t[m][:, r0:r0 + nr, :],
                            psm[:, 0:nr * OSP],
                            RELU, bias=bias(m))
                ss_all[s] = ss_t

            def xcorr_heads(s):
                ss_t = ss_all.pop(s)
                # xcorr: kernel rows 0-2 (15 taps) as diagonal matmuls on
                # the tensor engine; rows 3-4 (10 taps) as fused
                # multiply-accumulate chains on the vector engine.  For the
                # first and last samples (no neighbor work to hide the DVE
                # chain) all 25 taps run on the tensor engine instead.
                # taps 0..NTAP_PE-1 on PE, rest on VectorE; edge samples
                # lean on the PE (no neighbor work hides the DVE chain)
                NTAP_PE = 15 if s in (0, NB - 1) else 12
                ft_t = [fpool.tile([128, OX, OXP], F32R, tag=f"ft{j}",
                                   name=f"ft{j}_{s}") for j in range(2)]
                for j in range(2):
                    NT = OK * OKP
                    NDG = (OK - 2) * OKP
                    dg = dpool.tile([128, NDG, 128], F32R, tag=f"dg{j}",
                                    name=f"dg{j}_{s}")
                    # diagonals built on ScalarE (Copy with per-partition
                    # scale), only for the taps the PE runs
                    for t in range(NTAP_PE):
                        ky, kx = divmod(t, OK)
                        col = ky * OKP + kx
                        nc.scalar.activation(
                            dg[:, col, :], id_t[:], COPY,
                            scale=kf_t[j][:, s * NT + col:s * NT + col + 1])

                    acc = fpool.tile([128, OX, OXP], F32, tag=f"ac{j}",
                                     name=f"ac{j}_{s}")
                    first_acc = True
                    for t in range(NTAP_PE, OK * OK):
                        ky, kx = divmod(t, OK)
                        win = ss_t[j][:, ky:ky + OX, kx:kx + OXP]
                        kcol = kf_t[j][:, s * NT + ky * OKP + kx:
                                       s * NT + ky * OKP + kx + 1]
                        if first_acc:
                            nc.vector.tensor_scalar_mul(acc[:], win, kcol)
                            first_acc = False
                        else:
                            nc.vector.scalar_tensor_tensor(
                                acc[:], win, kcol, acc[:],
                                mybir.AluOpType.mult, mybir.AluOpType.add)

                    for ci, (r0, nr) in enumerate(X_CHUNKS):
                        psx = ps_x.tile([128, 512], F32, tag="px",
                                        name=f"psx{j}_{ci}_{s}")
                        for t in range(NTAP_PE):
                            ky, kx = divmod(t, OK)
                            rhs = ss_t[j][:, r0 + ky:r0 + ky + nr,
                                          kx:kx + OXP]
                            nc.tensor.matmul(
                                psx[:, 0:nr * OXP], dg[:, ky * OKP + kx, :],
                                rhs, start=(t == 0),
                                stop=(t == NTAP_PE - 1))
                        nc.vector.tensor_tensor(
                            ft_t[j][:, r0:r0 + nr, :],
                            psx[:, 0:nr * OXP].rearrange(
                                "p (a b) -> p a b", a=nr, b=OXP),
                            acc[:, r0:r0 + nr, :],
                            mybir.AluOpType.add)

                # 1x1 heads
                f2_t = [fpool.tile([128, OX, OXP], F32R, tag=f"f2{m}",
                                   name=f"f2{m}_{s}") for m in range(2)]
                for m in range(2):
                    for ci, (r0, nr) in enumerate(X_CHUNKS):
                        ps1 = ps_h.tile([128, 512], F32, tag="ph",
                                        name=f"ps1{m}_{ci}_{s}")
                        for j in range(2):
                            nc.tensor.matmul(
                                ps1[:, 0:nr * OXP],
                                w1_t[:, j, m * 128:(m + 1) * 128],
                                ft_t[j][:, r0:r0 + nr, :],
                                start=(j == 0), stop=(j == 1))
                        nc.scalar.activation(
                            f2_t[m][:, r0:r0 + nr, :],
                            ps1[:, 0:nr * OXP],
                            RELU, bias=bias(4 + m))

                for m in range(2):
                    # flat output tile -> fully contiguous DMA to DRAM
                    out_t = opool.tile([128, NPIX], F32, tag=f"o{m}",
                                       name=f"o{m}_{s}")
                    for ci, (r0, nr) in enumerate(X_CHUNKS):
                        ps2 = ps_h.tile([128, 512], F32, tag="ph",
                                        name=f"ps2{m}_{ci}_{s}")
                        for j in range(2):
                            nc.tensor.matmul(
                                ps2[:, 0:nr * OXP],
                                w2_t[:, j, m * 128:(m + 1) * 128],
                                f2_t[j][:, r0:r0 + nr, :],
                                start=(j == 0), stop=(j == 1))
                        nc.scalar.activation(
                            out_t[:, r0 * OX:(r0 + nr) * OX],
                            ps2[:, 0:nr * OXP].rearrange(
                                "p (a b) -> p a b", a=nr, b=OXP)[:, :, 0:OX],
                            IDENT, bias=bias(6 + m))
                        # per-chunk store: the exit drain only waits on the
                        # last small transfer
                        nc.sync.dma_start(
                            y_d[s, m * 128:(m + 1) * 128,
                                r0 * OX:(r0 + nr) * OX],
                            out_t[:, r0 * OX:(r0 + nr) * OX])

            for s in range(NB):
                search_conv(s)
                if s == 0:
                    kernel_conv()
                if s >= 1:
                    xcorr_heads(s - 1)
            xcorr_heads(NB - 1)

    nc.compile()
    return nc


def _get_nc():
    if "nc" not in _CACHED:
        _CACHED["nc"] = _build_nc()
    return _CACHED["nc"]


def _fold_bn(w, g, b, m, v):
    scale = g / np.sqrt(v + EPS)
    return w * scale[:, None, None, None], (b - m * scale)


def _pack3x3(w):
    t = w.transpose(2, 3, 1, 0).reshape(9, 2, 128, 256)  # t, j, p, c
    return np.ascontiguousarray(t.transpose(2, 0, 1, 3).astype(np.float32))


def _pack1x1(w):
    t = w[:, :, 0, 0].T.reshape(2, 128, 256)             # j, p, c
    return np.ascontiguousarray(t.transpose(1, 0, 2).astype(np.float32))


def _make_in_maps(kernel, search, wk, gk, bk, mk, vk, ws, gs, bs, ms, vs,
                  w1, g1, b1, m1, v1, w2, b2):
    wk_f, bk_f = _fold_bn(wk, gk, bk, mk, vk)
    ws_f, bs_f = _fold_bn(ws, gs, bs, ms, vs)
    w1_f, b1_f = _fold_bn(w1, g1, b1, m1, v1)

    xs = np.zeros((B, CIN, SS, SSP), np.float32)
    xs[:, :, :, :SS] = search
    xkp = np.zeros((B, CIN, SK, SKP), np.float32)
    xkp[:, :, :, :SK] = kernel
    # [2, 128, NB*70]: partition line holds all samples of one core
    xkp = xkp.reshape(B, CIN, SK * SKP)

    # bias columns: [bs0, bs1, bk0, bk1, b10, b11, b20, b21]
    bb = np.stack([bs_f[:128], bs_f[128:], bk_f[:128], bk_f[128:],
                   b1_f[:128], b1_f[128:],
                   np.asarray(b2)[:128], np.asarray(b2)[128:]],
                  axis=1).astype(np.float32)

    common = dict(
        ws=_pack3x3(ws_f), wk=_pack3x3(wk_f),
        w1=_pack1x1(w1_f), w2=_pack1x1(w2),
        bb=np.ascontiguousarray(bb),
        id128=np.eye(128, dtype=np.float32),
    )
    in_maps = []
    for c in range(NCORES):
        sl = slice(c * NB, (c + 1) * NB)
        xk_core = xkp[sl].reshape(NB, 2, 128, SK * SKP)
        xk_core = np.ascontiguousarray(
            xk_core.transpose(1, 2, 0, 3).reshape(2, 128, NB * SK * SKP))
        in_maps.append(dict(xs=np.ascontiguousarray(xs[sl]),
                            xk=xk_core, **common))
    return in_maps


def kernel(**inputs):
    in_maps = _make_in_maps(**inputs)
    nc = _get_nc()
    res = run_bass_kernel_spmd(nc, in_maps, core_ids=list(range(NCORES)))
    out = np.concatenate([r["y"] for r in res.results], axis=0)
    return out.reshape(B, COUT, OX, OX).astype(np.float32)
